# revision 9
# baseline (speedup 1.0000x reference)
"""ConsecutiveMatch kernel for Trainium2 (8 NeuronCores).

Reference semantics (per consecutive frame pair t, t+1):
    pcos[i, j] = cos(desc[t, i], desc[t+1, j])          # [N, N]
    confidence[t, i] = max_j pcos[i, j]
    idx[t, i] = argmax_j pcos[i, j]
    matched[t, i, :] = points[t+1, idx[t, i], :]

Sharding: data-parallel over the T-1=7 frame pairs; core c handles pair c
(core 7 duplicates pair 0, output discarded).

Device algorithm per core:
  - inputs arrive host-transposed: srcT/dstT [D=256, N=2048] as [128, 2, 2048]
    (partition = D%128, middle = D//128 chunk, free = keypoint index).
  - dst column norms via Square activation + ones-matmul (contract over
    partitions), then 1/max(sqrt(ss), eps) and a broadcast multiply so the
    matmul sees normalized dst. src is left raw: row norms only rescale each
    output row, so they cannot change the argmax; confidence is fixed up at
    the end by multiplying with 1/max(|src_i|, eps).
  - exact-fp32-class matmul at 3 cycles/row via the float32r hi/lo split:
    hi = round_f32r(x), lo = x - hi (exact), S = hi.hi' + hi.lo' + lo.hi'
    accumulated in fp32 PSUM. Dropped lo.lo' term is ~1e-9 of |S|.
  - per 128-row tile: DVE max (top-8) + max_index straight from PSUM give the
    row max and its first-occurrence index (matches jnp.argmax).

The host gathers matched points from the returned indices (pure indexing).
"""
import sys

sys.path.insert(0, "/opt/trn_rl_repo")

import numpy as np

T, N, D = 8, 2048, 256
NPAIR = T - 1
NCORES = 8
P = 128
KC = D // P            # 2 contraction chunks
NT = N // P            # 16 src row tiles
EPS = 1e-8

_compiled = None


def _build():
    import concourse.bacc as bacc
    import concourse.mybir as mybir
    from concourse.tile import TileContext

    F32 = mybir.dt.float32
    F32R = mybir.dt.float32r
    U32 = mybir.dt.uint32

    nc = bacc.Bacc(trn_type="TRN2", target_bir_lowering=False, debug=False)
    SRC = nc.dram_tensor("srcT", [P, KC, N], F32, kind="ExternalInput")
    DST = nc.dram_tensor("dstT", [P, KC, N], F32, kind="ExternalInput")
    CONF = nc.dram_tensor("conf", [P, NT], F32, kind="ExternalOutput")
    IDX = nc.dram_tensor("idx", [P, NT], U32, kind="ExternalOutput")


    with TileContext(nc) as tc:
        with (
            tc.tile_pool(name="big", bufs=1) as big,
            tc.tile_pool(name="small", bufs=1) as small,
            tc.tile_pool(name="dram", bufs=1, space="DRAM") as dram,
        ):
            INVD = dram.tile([1, N], F32)
            SSD = dram.tile([2, N], F32)
            srcT = big.tile([P, KC, N], F32)
            dstT = big.tile([P, KC, N], F32)
            nc.sync.dma_start(srcT, SRC.ap())
            nc.sync.dma_start(dstT, DST.ap())

            # ---- column sum-of-squares for src and dst via ones-matmul ----
            ones = small.tile([P, 1], F32)
            nc.vector.memset(ones, 1.0)
            sq = big.tile([P, KC, N], F32, name="sq")
            ss_sb = small.tile([1, 2, N], F32)  # [., 0, :]: dst, [., 1, :]: src
            with tc.tile_pool(name="npsum", bufs=1, space="PSUM") as npsum:
                for row, mat in ((0, dstT), (1, srcT)):
                    nc.scalar.activation(sq, mat, mybir.ActivationFunctionType.Square)
                    for n in range(4):
                        sp = npsum.tile([1, 512], F32, name="nsp")
                        for k in range(KC):
                            nc.tensor.matmul(
                                sp,
                                lhsT=ones,
                                rhs=sq[:, k, n * 512:(n + 1) * 512],
                                start=(k == 0),
                                stop=(k == KC - 1),
                            )
                        nc.vector.tensor_copy(ss_sb[:, row, n * 512:(n + 1) * 512], sp)

            # ---- redistribute [2, N] -> [128, 2, NT] via DRAM roundtrip ----
            # nrm2d[p, row, st] = ss[row, st*128 + p]
            nc.sync.dma_start(SSD.unsqueeze(0), ss_sb)
            nrm2d = small.tile([P, 2, NT], F32)
            nc.sync.dma_start(
                nrm2d,
                SSD.rearrange("r (st p) -> p r st", p=P),
            )
            nrm = small.tile([P, 2, NT], F32)
            nc.scalar.activation(nrm, nrm2d, mybir.ActivationFunctionType.Sqrt)
            nc.vector.tensor_scalar_max(nrm, nrm, EPS)
            inv = small.tile([P, 2, NT], F32)
            nc.vector.reciprocal(inv, nrm)

            # ---- scatter dst inverse norms back to [1, N] and broadcast ----
            nc.sync.dma_start(
                INVD.rearrange("one (st p) -> p one st", p=P),
                inv[:, 0:1, :],
            )
            invb = big.tile([P, N], F32, name="invb")
            nc.sync.dma_start(invb, INVD.broadcast_to([P, N]))

            # ---- normalize dst columns, then hi/lo split of both operands ----
            dstn = big.tile([P, KC, N], F32, name="dstn")
            for k in range(KC):
                nc.gpsimd.tensor_mul(dstn[:, k, :], dstT[:, k, :], invb)

            src_hi = big.tile([P, KC, N], F32R)
            src_lo = big.tile([P, KC, N], F32R)
            dst_hi = big.tile([P, KC, N], F32R)
            dst_lo = big.tile([P, KC, N], F32R)
            nc.scalar.copy(src_hi, srcT)
            nc.scalar.copy(dst_hi, dstn)
            nc.vector.tensor_sub(src_lo, srcT, src_hi)
            nc.vector.tensor_sub(dst_lo, dstn, dst_hi)

            # ---- per src-tile: 3-term matmul + max/max_index ----
            mx_all = small.tile([P, NT, 8], F32)
            ix_all = small.tile([P, NT, 8], U32)
            with tc.tile_pool(name="spsum", bufs=2, space="PSUM") as spsum:
                for st in range(NT):
                    sp = spsum.tile([P, N], F32, name="sp")
                    terms = ((src_hi, dst_hi), (src_hi, dst_lo), (src_lo, dst_hi))
                    for n in range(4):
                        for ti, (ta, tb) in enumerate(terms):
                            for k in range(KC):
                                nc.tensor.matmul(
                                    sp[:, n * 512:(n + 1) * 512],
                                    lhsT=ta[:, k, st * P:(st + 1) * P],
                                    rhs=tb[:, k, n * 512:(n + 1) * 512],
                                    start=(ti == 0 and k == 0),
                                    stop=(ti == 2 and k == KC - 1),
                                )
                    nc.vector.max(out=mx_all[:, st, :], in_=sp)
                    nc.vector.max_index(out=ix_all[:, st, :], in_max=mx_all[:, st, :], in_values=sp)

            # ---- confidence = rowmax / max(|src_i|, eps); emit outputs ----
            conf = small.tile([P, NT], F32)
            nc.vector.tensor_mul(conf, mx_all[:, :, 0], inv[:, 1, :])
            nc.sync.dma_start(CONF.ap(), conf)
            nc.sync.dma_start(IDX.ap(), ix_all[:, :, 0])

    nc.compile()
    return nc


def _get_compiled():
    global _compiled
    if _compiled is None:
        _compiled = _build()
    return _compiled


def kernel(descriptors: np.ndarray, points: np.ndarray):
    from concourse.bass_utils import run_bass_kernel_spmd

    descriptors = np.ascontiguousarray(descriptors, dtype=np.float32)
    points = np.ascontiguousarray(points, dtype=np.float32)

    # host-side layout prep: [T, N, D] -> [T, D//128 chunks...] transposed
    # descT[t] has shape [128, 2, N] with descT[t][p, k, j] = desc[t, j, k*128+p]
    descT = np.ascontiguousarray(
        descriptors.transpose(0, 2, 1).reshape(T, KC, P, N).transpose(0, 2, 1, 3)
    )

    nc = _get_compiled()
    in_maps = []
    for c in range(NCORES):
        t = c if c < NPAIR else 0
        in_maps.append({"srcT": descT[t], "dstT": descT[t + 1]})
    res = run_bass_kernel_spmd(nc, in_maps, core_ids=list(range(NCORES)))

    confidence = np.empty((NPAIR, N), dtype=np.float32)
    matched = np.empty((NPAIR, N, 2), dtype=np.float32)
    for t in range(NPAIR):
        out = res.results[t]
        conf2d = out["conf"]          # [128, NT] ; row index = st*128 + p
        idx2d = out["idx"]
        confidence[t] = conf2d.T.reshape(N)
        idx_full = idx2d.T.reshape(N).astype(np.int64)
        matched[t] = points[t + 1][idx_full]
    return matched, confidence


# revision 16
# speedup vs baseline: 1.0160x; 1.0160x over previous
"""ConsecutiveMatch kernel for Trainium2 (8 NeuronCores).

Reference semantics (per consecutive frame pair t, t+1):
    pcos[i, j] = cos(desc[t, i], desc[t+1, j])          # [N, N]
    confidence[t, i] = max_j pcos[i, j]
    matched[t, i, :] = points[t+1, argmax_j pcos[i, j], :]

Sharding: data-parallel over the T-1=7 frame pairs; core c handles pair c
(core 7 duplicates pair 0, output discarded).

Device algorithm per core (inputs arrive host-transposed as [128, 2, N],
partition = D%128, middle = D//128, free = keypoint):
  - dst column norms via Square activation + ones-matmul (contracting over
    partitions), 1/max(sqrt(ss), eps), then a broadcast multiply normalizes
    dst columns. src stays raw (row scaling can't change the argmax);
    confidence is fixed up at the end with 1/max(|src_i|, eps).
  - exact-fp32-class matmul at 3 cycles/row via the float32r hi/lo split:
    hi = round_f32r(x), lo = x - hi (exact), S = hi.hi' + hi.lo' + lo.hi'
    accumulated in fp32 PSUM (dropped lo.lo' term ~1e-9 of |S|).
  - per 128-row tile: DVE max (top-8) + max_index straight from PSUM give the
    row max and its first-occurrence argmax (matches jnp.argmax).

All front-end work is chunked into 4 column chunks with separate tiles per
chunk, so the first matmuls only depend on chunk 0 of the dst pipeline and
overlap the rest of the preprocessing.

The host gathers matched points from the returned indices (pure indexing).
"""
import sys

sys.path.insert(0, "/opt/trn_rl_repo")

import numpy as np

T, N, D = 8, 2048, 256
NPAIR = T - 1
NCORES = 8
P = 128
KC = D // P            # 2 contraction chunks
NT = N // P            # 16 src row tiles
CH = 4                 # column chunks for the front-end pipeline
CW = N // CH           # 512 columns per chunk
EPS = 1e-8

_compiled = None


def _build():
    import concourse.bacc as bacc
    import concourse.mybir as mybir
    from concourse.tile import TileContext

    F32 = mybir.dt.float32
    F32R = mybir.dt.float32r
    U32 = mybir.dt.uint32
    AF = mybir.ActivationFunctionType

    nc = bacc.Bacc(trn_type="TRN2", target_bir_lowering=False, debug=False)
    SRC = nc.dram_tensor("srcT", [P, KC, N], F32, kind="ExternalInput")
    DST = nc.dram_tensor("dstT", [P, KC, N], F32, kind="ExternalInput")
    CONF = nc.dram_tensor("conf", [P, NT], F32, kind="ExternalOutput")
    IDX = nc.dram_tensor("idx", [P, NT], U32, kind="ExternalOutput")

    with TileContext(nc) as tc:
        with (
            tc.tile_pool(name="big", bufs=1) as big,
            tc.tile_pool(name="small", bufs=1) as small,
            tc.tile_pool(name="sqp", bufs=2) as sqp,
            tc.tile_pool(name="dram", bufs=1, space="DRAM") as dram,
        ):
            INVD = dram.tile([1, N], F32)
            SSD_D = dram.tile([1, N], F32)
            SSD_S = dram.tile([1, N], F32)

            # chunked input loads on both HWDGE queues (SP + Activation)
            dst_c = [big.tile([P, KC, CW], F32, name=f"dstc{n}") for n in range(CH)]
            src_c = [big.tile([P, KC, CW], F32, name=f"srcc{n}") for n in range(CH)]
            for n in range(CH):
                nc.sync.dma_start(dst_c[n], DST.ap()[:, :, n * CW:(n + 1) * CW])
            for n in range(CH):
                nc.sync.dma_start(src_c[n], SRC.ap()[:, :, n * CW:(n + 1) * CW])

            ones = small.tile([P, P], F32)
            nc.vector.memset(ones, 1.0)

            def inv_norm_path(mats, ssd, tag):
                """sumsq per column -> [P, NT] inverse clamped norms."""
                ss = small.tile([1, N], F32, name=f"ss{tag}")
                for n in range(CH):
                    sq = sqp.tile([P, KC, CW], F32, name="sq")
                    nc.scalar.activation(sq, mats[n], AF.Square)
                    sp = npsum.tile([1, CW], F32, name="nsp")
                    for k in range(KC):
                        nc.tensor.matmul(
                            sp,
                            lhsT=ones,
                            rhs=sq[:, k, :],
                            start=(k == 0),
                            stop=(k == KC - 1),
                        )
                    nc.vector.tensor_copy(ss[:, n * CW:(n + 1) * CW], sp)
                # redistribute: nrm2d[p, st] = ss[0, st*128 + p]
                nc.sync.dma_start(ssd.unsqueeze(0), ss)
                nrm2d = small.tile([P, NT], F32, name=f"nrm2d{tag}")
                nc.sync.dma_start(nrm2d, ssd.rearrange("r (st p) -> p (r st)", p=P))
                nrm = small.tile([P, NT], F32, name=f"nrm{tag}")
                nc.scalar.activation(nrm, nrm2d, AF.Sqrt)
                nc.vector.tensor_scalar_max(nrm, nrm, EPS)
                inv = small.tile([P, NT], F32, name=f"inv{tag}")
                nc.vector.reciprocal(inv, nrm)
                return inv

            with tc.tile_pool(name="bpsum", bufs=2, space="PSUM") as bpsum:
                sh_c, sl_c, dh_c, dl_c = [], [], [], []
                for n in range(CH):
                    # column sumsq, replicated across partitions, in one shot:
                    # bc[m, j] = sum_k ones[k, m] * sq[k, j]
                    sq = sqp.tile([P, KC, CW], F32, name="sq")
                    nc.scalar.activation(sq, dst_c[n], AF.Square)
                    bc = bpsum.tile([P, CW], F32, name="bc")
                    for k in range(KC):
                        nc.tensor.matmul(
                            bc, lhsT=ones, rhs=sq[:, k, :],
                            start=(k == 0), stop=(k == KC - 1),
                        )
                    # norms: sqrt straight out of PSUM; reciprocal; normalize
                    nrmb = sqp.tile([P, CW], F32, name="nrmb")
                    nc.scalar.activation(nrmb, bc, AF.Sqrt)
                    invb = sqp.tile([P, CW], F32, name="invb")
                    nc.vector.reciprocal(invb, nrmb)
                    dstn = big.tile([P, KC, CW], F32, name=f"dstn{n}")
                    nc.gpsimd.tensor_mul(dstn[:, 0, :], dst_c[n][:, 0, :], invb)
                    nc.gpsimd.tensor_mul(dstn[:, 1, :], dst_c[n][:, 1, :], invb)
                    dh = big.tile([P, KC, CW], F32R, name=f"dh{n}")
                    dl = big.tile([P, KC, CW], F32R, name=f"dl{n}")
                    nc.scalar.copy(dh, dstn)
                    nc.vector.tensor_sub(dl, dstn, dh)
                    dh_c.append(dh)
                    dl_c.append(dl)

                # src hi/lo chunk 0 (tile 0 needs it); 1-3 emitted lazily
                def emit_src_hilo(n):
                    sh = big.tile([P, KC, CW], F32R, name=f"sh{n}")
                    sl = big.tile([P, KC, CW], F32R, name=f"sl{n}")
                    nc.scalar.copy(sh, src_c[n])
                    nc.vector.tensor_sub(sl, src_c[n], sh)
                    sh_c.append(sh)
                    sl_c.append(sl)

                emit_src_hilo(0)

            # ---- per src-tile: 3-term matmul + max/max_index ----
            mx_all = small.tile([P, NT, 8], F32)
            ix_all = small.tile([P, NT, 8], U32)
            with tc.tile_pool(name="spsum", bufs=2, space="PSUM") as spsum:
                for st in range(NT):
                    sc, co = st // 4, (st % 4) * P
                    if st % 4 == 0 and sc >= len(sh_c):
                        emit_src_hilo(sc)
                    sp = spsum.tile([P, N], F32, name="sp")
                    for n in range(CH):
                        terms = (
                            (sh_c[sc], dh_c[n]),
                            (sh_c[sc], dl_c[n]),
                            (sl_c[sc], dh_c[n]),
                        )
                        for ti, (ta, tb) in enumerate(terms):
                            for k in range(KC):
                                nc.tensor.matmul(
                                    sp[:, n * CW:(n + 1) * CW],
                                    lhsT=ta[:, k, co:co + P],
                                    rhs=tb[:, k, :],
                                    start=(ti == 0 and k == 0),
                                    stop=(ti == 2 and k == KC - 1),
                                )
                    nc.vector.max(out=mx_all[:, st, :], in_=sp)
                    nc.vector.max_index(
                        out=ix_all[:, st, :], in_max=mx_all[:, st, :], in_values=sp
                    )

            # ---- src norms (psum-free): Square -> k-add -> partition allreduce ----
            import concourse.bass_isa as bass_isa
            for n in range(CH):
                sq = sqp.tile([P, KC, CW], F32, name="sq")
                nc.scalar.activation(sq, src_c[n], AF.Square)
                sqk = sqp.tile([P, CW], F32, name="sqk")
                nc.gpsimd.tensor_add(sqk, sq[:, 0, :], sq[:, 1, :])
                ssr = sqp.tile([P, CW], F32, name="ssr")
                nc.gpsimd.partition_all_reduce(ssr, sqk, channels=P, reduce_op=bass_isa.ReduceOp.add)
                nc.sync.dma_start(SSD_S[:, n * CW:(n + 1) * CW], ssr[0:1, :])
            nrm2d_s = small.tile([P, NT], F32)
            nc.sync.dma_start(nrm2d_s, SSD_S.rearrange("r (st p) -> p (r st)", p=P))
            nrm_s = small.tile([P, NT], F32)
            nc.scalar.activation(nrm_s, nrm2d_s, AF.Sqrt)
            nc.vector.tensor_scalar_max(nrm_s, nrm_s, EPS)
            inv_s = small.tile([P, NT], F32)
            nc.vector.reciprocal(inv_s, nrm_s)

            # ---- confidence = rowmax * 1/max(|src_i|, eps); outputs ----
            conf = small.tile([P, NT], F32)
            nc.vector.tensor_mul(conf, mx_all[:, :, 0], inv_s)
            nc.sync.dma_start(CONF.ap(), conf)
            nc.sync.dma_start(IDX.ap(), ix_all[:, :, 0])

    nc.compile()
    return nc


def _get_compiled():
    global _compiled
    if _compiled is None:
        _compiled = _build()
    return _compiled


def kernel(descriptors: np.ndarray, points: np.ndarray):
    from concourse.bass_utils import run_bass_kernel_spmd

    descriptors = np.ascontiguousarray(descriptors, dtype=np.float32)
    points = np.ascontiguousarray(points, dtype=np.float32)

    # host-side layout prep: descT[t][p, k, j] = desc[t, j, k*128+p]
    descT = np.ascontiguousarray(
        descriptors.transpose(0, 2, 1).reshape(T, KC, P, N).transpose(0, 2, 1, 3)
    )

    nc = _get_compiled()
    in_maps = []
    for c in range(NCORES):
        t = c if c < NPAIR else 0
        in_maps.append({"srcT": descT[t], "dstT": descT[t + 1]})
    res = run_bass_kernel_spmd(nc, in_maps, core_ids=list(range(NCORES)))

    confidence = np.empty((NPAIR, N), dtype=np.float32)
    matched = np.empty((NPAIR, N, 2), dtype=np.float32)
    for t in range(NPAIR):
        out = res.results[t]
        confidence[t] = out["conf"].T.reshape(N)
        idx_full = out["idx"].T.reshape(N).astype(np.int64)
        matched[t] = points[t + 1][idx_full]
    return matched, confidence


# revision 19
# speedup vs baseline: 6119.5437x; 6023.4026x over previous
"""ConsecutiveMatch kernel for Trainium2 (8 NeuronCores).

Reference semantics (per consecutive frame pair t, t+1):
    pcos[i, j] = cos(desc[t, i], desc[t+1, j])          # [N, N]
    confidence[t, i] = max_j pcos[i, j]
    matched[t, i, :] = points[t+1, argmax_j pcos[i, j], :]

Sharding: data-parallel over the T-1=7 frame pairs; core c handles pair c
(core 7 duplicates pair 0, output discarded).

Device algorithm per core (inputs arrive host-transposed as [128, 2, N],
partition = D%128, middle = D//128, free = keypoint):
  - dst column norms via Square activation + ones-matmul (contracting over
    partitions), 1/max(sqrt(ss), eps), then a broadcast multiply normalizes
    dst columns. src stays raw (row scaling can't change the argmax);
    confidence is fixed up at the end with 1/max(|src_i|, eps).
  - exact-fp32-class matmul at 3 cycles/row via the float32r hi/lo split:
    hi = round_f32r(x), lo = x - hi (exact), S = hi.hi' + hi.lo' + lo.hi'
    accumulated in fp32 PSUM (dropped lo.lo' term ~1e-9 of |S|).
  - per 128-row tile: DVE max (top-8) + max_index straight from PSUM give the
    row max and its first-occurrence argmax (matches jnp.argmax).

All front-end work is chunked into 4 column chunks with separate tiles per
chunk, so the first matmuls only depend on chunk 0 of the dst pipeline and
overlap the rest of the preprocessing.

The host gathers matched points from the returned indices (pure indexing).
"""
import sys

sys.path.insert(0, "/opt/trn_rl_repo")

import numpy as np

T, N, D = 8, 2048, 256
NPAIR = T - 1
NCORES = 8
P = 128
KC = D // P            # 2 contraction chunks
NT = N // P            # 16 src row tiles
CH = 4                 # column chunks for the front-end pipeline
CW = N // CH           # 512 columns per chunk
EPS = 1e-8

_compiled = None


def _build():
    import concourse.bacc as bacc
    import concourse.mybir as mybir
    from concourse.tile import TileContext

    F32 = mybir.dt.float32
    F32R = mybir.dt.float32r
    U32 = mybir.dt.uint32
    AF = mybir.ActivationFunctionType

    nc = bacc.Bacc(trn_type="TRN2", target_bir_lowering=False, debug=False)
    SRC = nc.dram_tensor("srcT", [P, KC, N], F32, kind="ExternalInput")
    DST = nc.dram_tensor("dstT", [P, KC, N], F32, kind="ExternalInput")
    CONF = nc.dram_tensor("conf", [P, NT], F32, kind="ExternalOutput")
    IDX = nc.dram_tensor("idx", [P, NT], U32, kind="ExternalOutput")

    with TileContext(nc) as tc:
        with (
            tc.tile_pool(name="big", bufs=1) as big,
            tc.tile_pool(name="small", bufs=1) as small,
            tc.tile_pool(name="sqp", bufs=2) as sqp,
            tc.tile_pool(name="dram", bufs=1, space="DRAM") as dram,
        ):
            INVD = dram.tile([1, N], F32)
            SSD_D = dram.tile([1, N], F32)
            SSD_S = dram.tile([1, N], F32)

            # chunked input loads on both HWDGE queues (SP + Activation)
            dst_c = [big.tile([P, KC, CW], F32, name=f"dstc{n}") for n in range(CH)]
            src_c = [big.tile([P, KC, CW], F32, name=f"srcc{n}") for n in range(CH)]
            for n in range(CH):
                nc.sync.dma_start(dst_c[n], DST.ap()[:, :, n * CW:(n + 1) * CW])
            for n in range(CH):
                nc.sync.dma_start(src_c[n], SRC.ap()[:, :, n * CW:(n + 1) * CW])

            ones = small.tile([P, P], F32)
            nc.vector.memset(ones, 1.0)

            def inv_norm_path(mats, ssd, tag):
                """sumsq per column -> [P, NT] inverse clamped norms."""
                ss = small.tile([1, N], F32, name=f"ss{tag}")
                for n in range(CH):
                    sq = sqp.tile([P, KC, CW], F32, name="sq")
                    nc.scalar.activation(sq, mats[n], AF.Square)
                    sp = npsum.tile([1, CW], F32, name="nsp")
                    for k in range(KC):
                        nc.tensor.matmul(
                            sp,
                            lhsT=ones,
                            rhs=sq[:, k, :],
                            start=(k == 0),
                            stop=(k == KC - 1),
                        )
                    nc.vector.tensor_copy(ss[:, n * CW:(n + 1) * CW], sp)
                # redistribute: nrm2d[p, st] = ss[0, st*128 + p]
                nc.sync.dma_start(ssd.unsqueeze(0), ss)
                nrm2d = small.tile([P, NT], F32, name=f"nrm2d{tag}")
                nc.sync.dma_start(nrm2d, ssd.rearrange("r (st p) -> p (r st)", p=P))
                nrm = small.tile([P, NT], F32, name=f"nrm{tag}")
                nc.scalar.activation(nrm, nrm2d, AF.Sqrt)
                nc.vector.tensor_scalar_max(nrm, nrm, EPS)
                inv = small.tile([P, NT], F32, name=f"inv{tag}")
                nc.vector.reciprocal(inv, nrm)
                return inv

            with (
                tc.tile_pool(name="npsum", bufs=2, space="PSUM") as npsum,
                tc.tile_pool(name="bpsum", bufs=2, space="PSUM") as bpsum,
            ):
                sh_c, sl_c, dh_c, dl_c = [], [], [], []
                for n in range(CH):
                    # column sumsq of chunk n -> [1, CW] in PSUM
                    sq = sqp.tile([P, KC, CW], F32, name="sq")
                    nc.scalar.activation(sq, dst_c[n], AF.Square)
                    sp = npsum.tile([1, CW], F32, name="nsp")
                    for k in range(KC):
                        nc.tensor.matmul(
                            sp, lhsT=ones[:, :1], rhs=sq[:, k, :],
                            start=(k == 0), stop=(k == KC - 1),
                        )
                    ssb = sqp.tile([1, CW], F32, name="ssb")
                    nc.vector.tensor_copy(ssb, sp)
                    # broadcast across partitions via K=1 ones outer product
                    bc = bpsum.tile([P, CW], F32, name="bc")
                    nc.tensor.matmul(bc, lhsT=ones[:1, :], rhs=ssb)
                    # norms: sqrt straight out of PSUM; reciprocal; normalize
                    nrmb = sqp.tile([P, CW], F32, name="nrmb")
                    nc.scalar.activation(nrmb, bc, AF.Sqrt)
                    invb = sqp.tile([P, CW], F32, name="invb")
                    nc.vector.reciprocal(invb, nrmb)
                    dstn = big.tile([P, KC, CW], F32, name=f"dstn{n}")
                    nc.gpsimd.tensor_mul(dstn[:, 0, :], dst_c[n][:, 0, :], invb)
                    nc.gpsimd.tensor_mul(dstn[:, 1, :], dst_c[n][:, 1, :], invb)
                    dh = big.tile([P, KC, CW], F32R, name=f"dh{n}")
                    dl = big.tile([P, KC, CW], F32R, name=f"dl{n}")
                    nc.scalar.copy(dh, dstn)
                    nc.vector.tensor_sub(dl, dstn, dh)
                    dh_c.append(dh)
                    dl_c.append(dl)

                # src hi/lo chunk 0 (tile 0 needs it); 1-3 emitted lazily
                def emit_src_hilo(n):
                    sh = big.tile([P, KC, CW], F32R, name=f"sh{n}")
                    sl = big.tile([P, KC, CW], F32R, name=f"sl{n}")
                    nc.scalar.copy(sh, src_c[n])
                    nc.vector.tensor_sub(sl, src_c[n], sh)
                    sh_c.append(sh)
                    sl_c.append(sl)

                emit_src_hilo(0)

            # ---- per src-tile: 3-term matmul + max/max_index ----
            mx_all = small.tile([P, NT, 8], F32)
            ix_all = small.tile([P, NT, 8], U32)
            with tc.tile_pool(name="spsum", bufs=2, space="PSUM") as spsum:
                for st in range(NT):
                    sc, co = st // 4, (st % 4) * P
                    if st % 4 == 0 and sc >= len(sh_c):
                        emit_src_hilo(sc)
                    sp = spsum.tile([P, N], F32, name="sp")
                    for n in range(CH):
                        terms = (
                            (sh_c[sc], dh_c[n]),
                            (sh_c[sc], dl_c[n]),
                            (sl_c[sc], dh_c[n]),
                        )
                        for ti, (ta, tb) in enumerate(terms):
                            for k in range(KC):
                                nc.tensor.matmul(
                                    sp[:, n * CW:(n + 1) * CW],
                                    lhsT=ta[:, k, co:co + P],
                                    rhs=tb[:, k, :],
                                    start=(ti == 0 and k == 0),
                                    stop=(ti == 2 and k == KC - 1),
                                )
                    nc.vector.max(out=mx_all[:, st, :], in_=sp)
                    nc.vector.max_index(
                        out=ix_all[:, st, :], in_max=mx_all[:, st, :], in_values=sp
                    )

            # ---- src norms (psum-free): Square -> k-add -> partition allreduce ----
            import concourse.bass_isa as bass_isa
            for n in range(CH):
                sq = sqp.tile([P, KC, CW], F32, name="sq")
                nc.scalar.activation(sq, src_c[n], AF.Square)
                sqk = sqp.tile([P, CW], F32, name="sqk")
                nc.gpsimd.tensor_add(sqk, sq[:, 0, :], sq[:, 1, :])
                ssr = sqp.tile([P, CW], F32, name="ssr")
                nc.gpsimd.partition_all_reduce(ssr, sqk, channels=P, reduce_op=bass_isa.ReduceOp.add)
                nc.sync.dma_start(SSD_S[:, n * CW:(n + 1) * CW], ssr[0:1, :])
            nrm2d_s = small.tile([P, NT], F32)
            nc.sync.dma_start(nrm2d_s, SSD_S.rearrange("r (st p) -> p (r st)", p=P))
            nrm_s = small.tile([P, NT], F32)
            nc.scalar.activation(nrm_s, nrm2d_s, AF.Sqrt)
            nc.vector.tensor_scalar_max(nrm_s, nrm_s, EPS)
            inv_s = small.tile([P, NT], F32)
            nc.vector.reciprocal(inv_s, nrm_s)

            # ---- confidence = rowmax * 1/max(|src_i|, eps); outputs ----
            conf = small.tile([P, NT], F32)
            nc.vector.tensor_mul(conf, mx_all[:, :, 0], inv_s)
            nc.sync.dma_start(CONF.ap(), conf)
            nc.sync.dma_start(IDX.ap(), ix_all[:, :, 0])

    nc.compile()
    return nc


def _get_compiled():
    global _compiled
    if _compiled is None:
        _compiled = _build()
    return _compiled


def kernel(descriptors: np.ndarray, points: np.ndarray):
    from concourse.bass_utils import run_bass_kernel_spmd

    descriptors = np.ascontiguousarray(descriptors, dtype=np.float32)
    points = np.ascontiguousarray(points, dtype=np.float32)

    # host-side layout prep: descT[t][p, k, j] = desc[t, j, k*128+p]
    descT = np.ascontiguousarray(
        descriptors.transpose(0, 2, 1).reshape(T, KC, P, N).transpose(0, 2, 1, 3)
    )

    nc = _get_compiled()
    in_maps = []
    for c in range(NCORES):
        t = c if c < NPAIR else 0
        in_maps.append({"srcT": descT[t], "dstT": descT[t + 1]})
    res = run_bass_kernel_spmd(nc, in_maps, core_ids=list(range(NCORES)))

    confidence = np.empty((NPAIR, N), dtype=np.float32)
    matched = np.empty((NPAIR, N, 2), dtype=np.float32)
    for t in range(NPAIR):
        out = res.results[t]
        confidence[t] = out["conf"].T.reshape(N)
        idx_full = out["idx"].T.reshape(N).astype(np.int64)
        matched[t] = points[t + 1][idx_full]
    return matched, confidence
